# revision 17
# baseline (speedup 1.0000x reference)
"""Causal self-attention (GQA + RoPE) Trainium2 Bass kernel.

Sharding: 8 cores = 2 (batch) x 4 (kv-head groups). Each core computes the
full attention for one batch element and one kv head (with its 4 query
heads), producing a partial output projection (row-split Wproj); the host
sums the 4 kv-group partials per batch element.

Self-contained: hardcodes B=2, T=2048, E=2048, H=16, HKV=4, D=128.
"""

import sys

for _p in ("/opt/trn_rl_repo", "/root/.axon_site/_ro/trn_rl_repo"):
    if _p not in sys.path:
        sys.path.append(_p)

import math
from contextlib import ExitStack

import numpy as np

import concourse.bacc as bacc
import concourse.tile as tile
import concourse.mybir as mybir
from concourse.bass_utils import run_bass_kernel_spmd

P = 128          # partitions
T = 2048         # sequence length
E = 2048         # embed dim
D = 128          # head dim
GH = 4           # query heads per core (= per kv head)
CH = 512         # t-chunk width (PSUM bank = 512 f32)
NCH = T // CH    # 4 t-chunks
NE = E // P      # 16 contraction chunks over E
NK = T // P      # 16 key tiles
NDIAG = CH // P  # 4 diagonal mask offsets

F32 = mybir.dt.float32
F32R = mybir.dt.float32r
F16 = mybir.dt.float16
EXPF = mybir.ActivationFunctionType.Exp


def _emit(nc):
    x = nc.dram_tensor("x", [T, E], F16, kind="ExternalInput")
    wq = nc.dram_tensor("wq", [E, GH * D], F16, kind="ExternalInput")
    wk = nc.dram_tensor("wk", [E, D], F16, kind="ExternalInput")
    wv = nc.dram_tensor("wv", [E, D], F16, kind="ExternalInput")
    wp = nc.dram_tensor("wp", [GH * D, E], F32, kind="ExternalInput")
    cos = nc.dram_tensor("cos", [D, T], F16, kind="ExternalInput")
    sn = nc.dram_tensor("sn", [D, T], F16, kind="ExternalInput")
    mask = nc.dram_tensor("mask", [P, NDIAG, CH], F16, kind="ExternalInput")
    ident = nc.dram_tensor("ident", [P, P], F16, kind="ExternalInput")
    y = nc.dram_tensor("y", [T, E], F32, kind="ExternalOutput")

    with tile.TileContext(nc) as tc, ExitStack() as ctx:
        # ---- persistent pools (live across phases) ----
        pool_cst = ctx.enter_context(tc.tile_pool(name="cst", bufs=1))
        pool_qfin = ctx.enter_context(tc.tile_pool(name="qfin", bufs=GH))
        pool_kfin = ctx.enter_context(tc.tile_pool(name="kfin", bufs=1))
        pool_vfin = ctx.enter_context(tc.tile_pool(name="vfin", bufs=1))

        ident16 = pool_cst.tile([P, P], F16)
        ones16 = pool_cst.tile([P, P], F16)
        nc.gpsimd.memset(ones16[:], 1.0)
        mask_sb = pool_cst.tile([P, NDIAG, CH], F16)

        qfin = [pool_qfin.tile([P, T], F16, tag="qfin", name=f"qfin{h}") for h in range(GH)]
        kfin = pool_kfin.tile([P, T], F16)
        vfin = pool_vfin.tile([P, NK, P], F16)

        def rope_combine(dst_slice, psrc, cos_sl, sn_sl, pool):
            # dst = psrc * cos + rotate_half(psrc) * sn   (sn has rows 0:64 negated)
            raw = pool.tile([P, CH], F16, tag="rp_raw")
            nc.scalar.copy(raw[:], psrc[:])
            sw = pool.tile([P, CH], F16, tag="rp_sw")
            nc.vector.tensor_copy(sw[0:64, :], raw[64:128, :])
            nc.vector.tensor_copy(sw[64:128, :], raw[0:64, :])
            m1 = pool.tile([P, CH], F16, tag="rp_m1")
            nc.vector.tensor_mul(m1[:], raw[:], cos_sl)
            nc.vector.tensor_mul(sw[:], sw[:], sn_sl)
            nc.vector.tensor_add(dst_slice, m1[:], sw[:])

        # ================= Phase B: projections + RoPE =================
        with (
            tc.tile_pool(name="wqp", bufs=1) as pool_wq,
            tc.tile_pool(name="wkv", bufs=1) as pool_wkv,
            tc.tile_pool(name="tab", bufs=1) as pool_tab,
            tc.tile_pool(name="xsl", bufs=4) as pool_xsl,
            tc.tile_pool(name="xtp", bufs=2 * NE) as pool_xt,
            tc.tile_pool(name="pstr", bufs=2, space="PSUM") as ps_tr,
            tc.tile_pool(name="rw", bufs=3) as pool_rw,
            tc.tile_pool(name="vts", bufs=1) as pool_vt,
            tc.tile_pool(name="pspj", bufs=1, space="PSUM") as ps_pj,
        ):
            vt_sb = pool_vt.tile([P, T], F16)

            def load_xt_chunk_xbar(c):
                tiles = []
                for e in range(NE):
                    xt = pool_xt.tile([P, CH], F16, tag="xt",
                                      name=f"xt{c}_{e}")
                    nc.sync.dma_start(
                        xt[:],
                        x.ap()[c * CH:(c + 1) * CH, e * P:(e + 1) * P],
                        transpose=True,
                    )
                    tiles.append(xt)
                return tiles

            # chunk 0: fast row loads + PE transposes (instant start)
            nc.sync.dma_start(ident16[:], ident.ap()[:])
            nc.sync.dma_start(mask_sb[:], mask.ap()[:])
            xrow0 = []
            for j in range(4):
                xr = pool_xsl.tile([P, E], F16, tag="xr", name=f"xr{j}")
                nc.sync.dma_start(xr[:], x.ap()[j * P:(j + 1) * P, :])
                xrow0.append(xr)

            # weights ordered so each lands when first needed
            wk_r = pool_wkv.tile([P, NE, D], F16, tag="wk")
            nc.sync.dma_start(
                wk_r[:], wk.ap().rearrange("(n p) m -> p n m", p=P))
            wv_r = pool_wkv.tile([P, NE, D], F16, tag="wv")
            nc.sync.dma_start(
                wv_r[:], wv.ap().rearrange("(n p) m -> p n m", p=P))
            wq_r = pool_wq.tile([P, NE, GH * D], F16)
            nc.sync.dma_start(
                wq_r[:], wq.ap().rearrange("(n p) m -> p n m", p=P))
            cos_sb = pool_tab.tile([P, T], F16, tag="cos")
            nc.sync.dma_start(cos_sb[:], cos.ap()[:])
            sn_sb = pool_tab.tile([P, T], F16, tag="sn")
            nc.sync.dma_start(sn_sb[:], sn.ap()[:])

            # chunk-0 PE transposes: one PSUM bank = 2 e-tiles x 4 t-tiles
            xts0 = []
            for e in range(NE):
                xt = pool_xt.tile([P, CH], F16, tag="xt", name=f"xt0_{e}")
                xts0.append(xt)
            for ep in range(NE // 2):
                bank = ps_tr.tile([P, 2 * CH], F16, tag="tr")
                for half in (0, 1):
                    e = 2 * ep + half
                    for j in range(4):
                        nc.tensor.matmul(
                            bank[:, half * CH + j * P: half * CH + (j + 1) * P],
                            xrow0[j][:, e * P:(e + 1) * P],
                            ident16[:],
                            is_transpose=True,
                            start=(half == 0 and j == 0),
                            stop=(half == 1 and j == 3),
                        )
                for half in (0, 1):
                    nc.vector.tensor_copy(
                        xts0[2 * ep + half][:],
                        bank[:, half * CH:(half + 1) * CH],
                    )

            pending_xt = {1: load_xt_chunk_xbar(1)}

            for c in range(NCH):
                if c == 0:
                    xts = xts0
                else:
                    xts = pending_xt.pop(c)
                if c + 2 < NCH:
                    pending_xt[c + 2] = load_xt_chunk_xbar(c + 2)

                # --- k/v/q projections, e-outer (rate-matched to xbar) ---
                pk = ps_pj.tile([P, CH], F32, tag="pk", bufs=1, name=f"pk{c}")
                pv = ps_pj.tile([P, CH], F32, tag="pv", bufs=1, name=f"pv{c}")
                pqs = [ps_pj.tile([P, CH], F32, tag=f"pq{h}", bufs=1,
                                  name=f"pq{c}_{h}") for h in range(GH)]
                for e in range(NE):
                    st = (e == 0)
                    sp = (e == NE - 1)
                    nc.tensor.matmul(pk[:], wk_r[:, e, :], xts[e][:],
                                     start=st, stop=sp)
                    nc.tensor.matmul(pv[:], wv_r[:, e, :], xts[e][:],
                                     start=st, stop=sp)
                    for h in range(GH):
                        nc.tensor.matmul(
                            pqs[h][:],
                            wq_r[:, e, h * D:(h + 1) * D],
                            xts[e][:],
                            start=st, stop=sp,
                        )
                rope_combine(
                    kfin[:, c * CH:(c + 1) * CH], pk,
                    cos_sb[:, c * CH:(c + 1) * CH],
                    sn_sb[:, c * CH:(c + 1) * CH],
                    pool_rw,
                )
                nc.scalar.copy(vt_sb[:, c * CH:(c + 1) * CH], pv[:])
                # --- v natural layout for this chunk's 4 k-tiles ---
                for j in range(4):
                    kt = c * 4 + j
                    nc.sync.dma_start(
                        vfin[:, kt, :],
                        vt_sb[:, kt * P:(kt + 1) * P],
                        transpose=True,
                    )
                for h in range(GH):
                    rope_combine(
                        qfin[h][:, c * CH:(c + 1) * CH], pqs[h],
                        cos_sb[:, c * CH:(c + 1) * CH],
                        sn_sb[:, c * CH:(c + 1) * CH],
                        pool_rw,
                    )

        # ================= Phase C: attention =================
        pool_wp_ = ctx.enter_context(tc.tile_pool(name="wpp", bufs=1))
        pool_outf = ctx.enter_context(tc.tile_pool(name="outf", bufs=GH))
        wp_r = pool_wp_.tile([P, GH, E], F32R)
        outf = [pool_outf.tile([P, T], F32R, tag="outf", name=f"outf{h}") for h in range(GH)]

        with (
            tc.tile_pool(name="wps", bufs=2) as pool_wps,
            tc.tile_pool(name="expb", bufs=4) as pool_exp,
            tc.tile_pool(name="attw", bufs=4) as pool_attw,
            tc.tile_pool(name="scps", bufs=2, space="PSUM") as ps_sc,
            tc.tile_pool(name="avps", bufs=2, space="PSUM") as ps_av,
            tc.tile_pool(name="smps", bufs=2, space="PSUM") as ps_sm,
        ):
            for h in range(GH):
                for c in range(NCH):
                    nk = 4 * c + 4
                    npair = nk // 2
                    av = ps_av.tile([P, CH], F32, tag="av")
                    sm = ps_sm.tile([P, CH], F32, tag="sm")
                    exps = {}
                    for kp in range(npair + 1):
                        if kp < npair:
                            sc = ps_sc.tile([P, 2 * CH], F32, tag="sc")
                            for half in (0, 1):
                                k = 2 * kp + half
                                nc.tensor.matmul(
                                    sc[:, half * CH:(half + 1) * CH],
                                    kfin[:, k * P:(k + 1) * P],
                                    qfin[h][:, c * CH:(c + 1) * CH],
                                    start=True,
                                    stop=True,
                                )
                            ex = pool_exp.tile([P, 2 * CH], F16, tag="ex")
                            nc.scalar.activation(ex[:], sc[:], EXPF)
                            for half in (0, 1):
                                k = 2 * kp + half
                                m = k - 4 * c
                                if m >= 0:
                                    nc.gpsimd.tensor_mul(
                                        ex[:, half * CH:(half + 1) * CH],
                                        ex[:, half * CH:(half + 1) * CH],
                                        mask_sb[:, m, :],
                                    )
                            exps[kp] = ex
                        if kp >= 1:
                            ex = exps.pop(kp - 1)
                            for half in (0, 1):
                                k = 2 * (kp - 1) + half
                                nc.tensor.matmul(
                                    av[:],
                                    vfin[:, k, :],
                                    ex[:, half * CH:(half + 1) * CH],
                                    start=(k == 0),
                                    stop=(k == nk - 1),
                                )
                                nc.tensor.matmul(
                                    sm[:],
                                    ones16[:],
                                    ex[:, half * CH:(half + 1) * CH],
                                    start=(k == 0),
                                    stop=(k == nk - 1),
                                )
                    rec = pool_attw.tile([P, CH], F32, tag="rec")
                    nc.vector.reciprocal_approx_fast(rec[:], sm[:])
                    nc.vector.tensor_mul(
                        outf[h][:, c * CH:(c + 1) * CH], av[:], rec[:]
                    )

            # wp load late (phase D input) so it never blocks attention DMAs
            for j in range(GH):
                stg = pool_wps.tile([P, E], F32, tag="wpstg")
                nc.sync.dma_start(stg[:], wp.ap()[j * P:(j + 1) * P, :])
                nc.vector.tensor_copy(wp_r[:, j, :], stg[:])

        # ================= Phase D: output projection =================
        with (
            tc.tile_pool(name="ystg", bufs=4) as pool_y,
            tc.tile_pool(name="pyps", bufs=4, space="PSUM") as ps_y,
        ):
            for t in range(NK):
                for eo2 in range(NCH // 2):
                    py = ps_y.tile([P, 2 * CH], F32, tag="py")
                    for half in (0, 1):
                        eo = 2 * eo2 + half
                        for j in range(GH):
                            nc.tensor.matmul(
                                py[:, half * CH:(half + 1) * CH],
                                outf[j][:, t * P:(t + 1) * P],
                                wp_r[:, j, eo * CH:(eo + 1) * CH],
                                start=(j == 0),
                                stop=(j == GH - 1),
                            )
                    ys = pool_y.tile([P, 2 * CH], F32, tag="ys")
                    nc.vector.tensor_copy(ys[:], py[:])
                    nc.sync.dma_start(
                        y.ap()[t * P:(t + 1) * P, 2 * eo2 * CH:(2 * eo2 + 2) * CH],
                        ys[:],
                    )

    return nc


_NC = None


def build_nc():
    global _NC
    if _NC is None:
        nc = bacc.Bacc("TRN2", target_bir_lowering=False, debug=False)
        _emit(nc)
        nc.compile()
        _NC = nc
    return _NC


def host_tables(pos):
    """RoPE tables, exactly mirroring the reference construction."""
    half = D // 2
    inv_freq = (1.0 / np.power(10000.0, np.arange(0, D, 2, dtype=np.float32) / D))
    t = np.arange(pos, pos + T, dtype=np.float32)
    freqs = t[:, None] * inv_freq[None, :]          # [T, half]
    freqs = np.repeat(freqs, 2, axis=-1)            # [T, D]
    cos = np.cos(freqs).astype(np.float32).T.copy() # [D, T]
    sin = np.sin(freqs).astype(np.float32).T.copy() # [D, T]
    sn = sin.copy()
    sn[:half] = -sn[:half]
    return (np.ascontiguousarray(cos).astype(np.float16),
            np.ascontiguousarray(sn).astype(np.float16))


def host_masks():
    kk = np.arange(P)[:, None]
    qq = np.arange(CH)[None, :]
    m = np.stack(
        [(kk + 128 * i <= qq) for i in range(NDIAG)], axis=1
    )  # [P, NDIAG, CH]
    return m.astype(np.float16)


def make_in_maps(x, Wq, Wk, Wv, Wproj, pos):
    x = np.asarray(x, dtype=np.float32)
    Wq = np.asarray(Wq, dtype=np.float32)
    Wk = np.asarray(Wk, dtype=np.float32)
    Wv = np.asarray(Wv, dtype=np.float32)
    Wproj = np.asarray(Wproj, dtype=np.float32)
    scale = np.float32(1.0 / math.sqrt(D))
    cos, sn = host_tables(int(pos))
    mask = host_masks()
    in_maps = []
    for c in range(8):
        b, g = divmod(c, 4)
        in_maps.append({
            "x": np.ascontiguousarray(x[b]).astype(np.float16),
            "wq": np.ascontiguousarray(
                Wq[:, g * GH * D:(g + 1) * GH * D] * scale).astype(np.float16),
            "wk": np.ascontiguousarray(Wk[:, g * D:(g + 1) * D]).astype(np.float16),
            "wv": np.ascontiguousarray(Wv[:, g * D:(g + 1) * D]).astype(np.float16),
            "wp": np.ascontiguousarray(Wproj[g * GH * D:(g + 1) * GH * D, :]),
            "cos": cos,
            "sn": sn,
            "mask": mask,
            "ident": np.eye(P, dtype=np.float16),
        })
    return in_maps


def kernel_with_results(x, Wq, Wk, Wv, Wproj, pos, trace=False):
    nc = build_nc()
    in_maps = make_in_maps(x, Wq, Wk, Wv, Wproj, pos)
    res = run_bass_kernel_spmd(nc, in_maps, list(range(8)), trace=trace)
    B = 2
    y = np.zeros((B, T, E), dtype=np.float32)
    for c in range(8):
        b = c // 4
        y[b] += res.results[c]["y"]
    return y, res


def kernel(x, Wq, Wk, Wv, Wproj, pos):
    y, _ = kernel_with_results(x, Wq, Wk, Wv, Wproj, pos)
    return y
